# revision 13
# baseline (speedup 1.0000x reference)
"""PhaseSimilarity Trainium2 kernel.

out[b,h,t,a] = where(valid, |sum_n exp(i*(q[b,t,n] - k[b,h,t,a,n]))| / N, -inf)

Strategy (8 NeuronCores, shard (b, t-quarter)):
  per core: 8 heads x 512 t x 64 a x 64 n = 16.7M k-elements (64 MiB fp32).
  1. gpsimd cast-DMA k fp32(HBM) -> fp16(SBUF), natural (t, (a n)) layout.
  2. xbar DMA-transpose -> (p=(a%2,n), J=a//2, t): contraction dim n on
     partitions.
  3. fp16 turns-domain range reduction on DVE (Sin on ScalarE needs
     [-pi,pi]): r' = k/(2pi) - q/(2pi); frac via magic-number rounding;
     separate quarter-shifted rounding for the cos path. Signs are free
     (real/imag get squared).
  4. ScalarE Sin -> cd, sd (fp16)  [the compute bottleneck: 2 passes].
  5. TensorE reduces over n: lhsT = cd-tile (128, t), rhs = 2-col parity
     selector -> PSUM (t, a) in natural output layout, accumulated across
     J with has_written semantics.
  6. Epilogue (batched at the end, 2 ACT table switches total):
     mag = Sqrt((r^2+i^2)/N^2), out = select(valid, mag, -inf).
"""
import numpy as np

B, H, T, A, N = 2, 8, 2048, 64, 64
NCORES = 8
TQS = 4                # t-quarters
TQ = T // TQS          # 512 t per core
PT = 128               # t-chunk (partition dim)
NCH = TQ // PT         # 4 chunks per core
NJ = A // 2            # 32 a-pairs
FREE = A * N           # 4096

INV2PI = 1.0 / (2 * np.pi)
MAGIC = 1536.0         # 1.5 * 2^10: fp16 round-to-nearest-int magic
TWOPI_S = 6.2831       # slightly < 2pi: |scale*frac| strictly inside pi

_CACHE = {}


def _build(loop_n: int = 1):
    import os
    PROBE = os.environ.get("PROBE", "")
    from contextlib import ExitStack

    import concourse.bacc as bacc
    import concourse.bass as bass
    import concourse.tile as tile
    from concourse import mybir

    f16 = mybir.dt.float16
    f32 = mybir.dt.float32
    u8 = mybir.dt.uint8
    SIN = mybir.ActivationFunctionType.Sin
    SQUARE = mybir.ActivationFunctionType.Square
    SQRT = mybir.ActivationFunctionType.Sqrt
    SUB = mybir.AluOpType.subtract
    MULT = mybir.AluOpType.mult
    ISGE = mybir.AluOpType.is_ge

    nc = bacc.Bacc(
        "TRN2", target_bir_lowering=False, debug=False,
        enable_asserts=False, num_devices=NCORES,
    )
    k_d = nc.dram_tensor("k", [H, TQ, A, N], f16, kind="ExternalInput").ap()
    q_d = nc.dram_tensor("q", [TQ, N], f32, kind="ExternalInput").ap()
    v_d = nc.dram_tensor("valid", [H, TQ, A], u8, kind="ExternalInput").ap()
    o_d = nc.dram_tensor("out", [H, TQ, A], f32, kind="ExternalOutput").ap()

    with tile.TileContext(nc) as tc, ExitStack() as ctx:
        # load/trig tiles double-buffered for cross-stage overlap; the
        # DVE-internal chain temps (rp, r, rc) are single-buffered (DVE is
        # serial with itself anyway).
        work = ctx.enter_context(tc.tile_pool(name="work", bufs=2))
        loadp = ctx.enter_context(tc.tile_pool(name="loadp", bufs=3))
        chain = ctx.enter_context(tc.tile_pool(name="chain", bufs=1))
        qpool = ctx.enter_context(tc.tile_pool(name="qp", bufs=2))
        acc = ctx.enter_context(tc.tile_pool(name="acc", bufs=1))
        psum = ctx.enter_context(tc.tile_pool(name="psum", bufs=4, space="PSUM"))

        # cos-path bias: Sin(-S*t + S/4)
        cbias = acc.tile([128, 1], f32)
        nc.vector.memset(cbias, TWOPI_S / 4)

        # parity selector: rows 0-63 -> col 0, rows 64-127 -> col 1
        sel = acc.tile([128, 2], f16)
        nc.vector.memset(sel, 0.0)
        nc.vector.memset(sel[0:64, 0:1], 1.0)
        nc.vector.memset(sel[64:128, 1:2], 1.0)

        # per-core accumulation buffers (128, NCH, H, A)
        rbuf = acc.tile([PT, NCH, H, A], f32)
        ibuf = acc.tile([PT, NCH, H, A], f32)
        mbuf = acc.tile([PT, NCH, H, A], u8)

        loop_cm = tc.For_i(0, loop_n, 1) if loop_n > 1 else None
        if loop_cm is not None:
            ctx.enter_context(loop_cm)

        for ch in range(NCH):
            t0 = ch * PT
            # q/(2pi) - MAGICless prep: load fp32, scale, duplicate, transpose
            qf32 = qpool.tile([PT, N], f32, tag="qf32")
            nc.sync.dma_start(out=qf32, in_=q_d[t0:t0 + PT, :])
            qf = qpool.tile([PT, 2, N], f16, tag="qf")
            nc.vector.tensor_scalar_mul(qf[:, 0, :], qf32, INV2PI)
            nc.vector.tensor_scalar_mul(qf[:, 1, :], qf32, INV2PI)
            qT = qpool.tile([128, PT], f16, tag="qT")
            nc.sync.dma_start_transpose(qT, qf.rearrange("t j n -> t (j n)"))
            # materialize q/(2pi) replicated over J (dense step-1 operand so
            # the stt ops hit the fp16 2x perf mode); reused across all 8 h.
            qrep = qpool.tile([128, NJ, PT], f16, tag="qrep")
            nc.vector.tensor_copy(qrep[:, 0, :], qT)
            ncopied = 1
            while ncopied < NJ:
                step = min(ncopied, NJ - ncopied)
                nc.vector.tensor_copy(
                    qrep[:, ncopied:ncopied + step, :], qrep[:, 0:step, :])
                ncopied += step

            for h in range(H):
                # xbar-transpose straight from DRAM (k pre-cast to fp16 on
                # host): lands (p=(a%2,n), J=a//2, t) with no SBUF staging.
                kT = loadp.tile([128, NJ, PT], f16, tag="kT")
                nc.sync.dma_start_transpose(
                    kT, k_d[h, t0:t0 + PT, :, :].rearrange("t a n -> t (a n)"))

                # range reduction (turns domain, fp16)
                rp = chain.tile([128, NJ, PT], f16, tag="rp")
                nc.vector.scalar_tensor_tensor(rp, kT, INV2PI, qrep,
                                               op0=MULT, op1=SUB)
                r = chain.tile([128, NJ, PT], f16, tag="r")
                nc.vector.tensor_scalar_add(r, rp, MAGIC)
                mfrac = work.tile([128, NJ, PT], f16, tag="mfrac")
                nc.vector.scalar_tensor_tensor(mfrac, r, MAGIC, rp,
                                               op0=SUB, op1=SUB)
                # cos arg: t = (mfrac >= 1/4) - mfrac; cos(d) = Sin(-S*t + S/4)
                fc = work.tile([128, NJ, PT], f16, tag="fc")
                nc.vector.scalar_tensor_tensor(fc, mfrac, 0.25, mfrac,
                                               op0=ISGE, op1=SUB)

                # trig (ScalarE): sd = -sin(d), cd = -cos(d); signs cancel
                sd = work.tile([128, NJ, PT], f16, tag="sd")
                cd = work.tile([128, NJ, PT], f16, tag="cd")
                if PROBE == "noact":
                    sd, cd = mfrac, fc
                else:
                    nc.scalar.activation(sd, mfrac, SIN, scale=TWOPI_S)
                    nc.scalar.activation(cd, fc, SIN, scale=-TWOPI_S, bias=cbias)

                # reduce over n: PSUM (t, a)
                ps_r = psum.tile([PT, A], f32, tag="ps_r")
                ps_i = psum.tile([PT, A], f32, tag="ps_i")
                NJ_MM = 1 if PROBE == "fewmm" else NJ
                for J in range(NJ_MM):
                    nc.tensor.matmul(ps_r[:, 2 * J:2 * J + 2], cd[:, J, :],
                                     sel, start=(J == 0), stop=(J == NJ_MM - 1))
                for J in range(NJ_MM):
                    nc.tensor.matmul(ps_i[:, 2 * J:2 * J + 2], sd[:, J, :],
                                     sel, start=(J == 0), stop=(J == NJ_MM - 1))

                nc.scalar.copy(rbuf[:, ch, h, :], ps_r)
                nc.scalar.copy(ibuf[:, ch, h, :], ps_i)
                nc.sync.dma_start(out=mbuf[:, ch, h, :],
                                  in_=v_d[h, t0:t0 + PT, :])

        # epilogue: mag = sqrt((r^2 + i^2)/N^2); out = valid ? mag : -inf
        # squares in-place (elementwise stream, writes trail reads)
        nc.scalar.activation(rbuf, rbuf, SQUARE)
        nc.scalar.activation(ibuf, ibuf, SQUARE)
        ss = acc.tile([PT, NCH, H, A], f32)
        nc.vector.tensor_add(ss, rbuf, ibuf)
        nc.scalar.activation(ss, ss, SQRT, scale=1.0 / (N * N))
        res = acc.tile([PT, NCH, H, A], f32)
        nc.vector.memset(res, float("-inf"))
        nc.vector.copy_predicated(res, mbuf, ss)
        for ch in range(NCH):
            for h in range(H):
                nc.sync.dma_start(out=o_d[h, ch * PT:(ch + 1) * PT, :],
                                  in_=res[:, ch, h, :])
    nc.compile()
    return nc


def kernel(q_angles, k_angles, valid, batch, time):
    from concourse.bass_utils import run_bass_kernel_spmd

    if "nc" not in _CACHE:
        _CACHE["nc"] = _build()
    nc = _CACHE["nc"]

    q_angles = np.asarray(q_angles, dtype=np.float32)
    k_angles = np.asarray(k_angles, dtype=np.float32)
    valid_u8 = np.asarray(valid).astype(np.uint8)

    in_maps = []
    for c in range(NCORES):
        b, tq = divmod(c, TQS)
        sl = slice(tq * TQ, (tq + 1) * TQ)
        in_maps.append({
            "k": np.ascontiguousarray(k_angles[b, :, sl]).astype(np.float16),
            "q": np.ascontiguousarray(q_angles[b, sl]),
            "valid": np.ascontiguousarray(valid_u8[b, :, sl]),
        })

    res = run_bass_kernel_spmd(nc, in_maps, core_ids=list(range(NCORES)))
    out = np.empty((B, H, T, A), dtype=np.float32)
    for c in range(NCORES):
        b, tq = divmod(c, TQS)
        out[b, :, tq * TQ:(tq + 1) * TQ] = res.results[c]["out"]
    return out


# revision 14
# speedup vs baseline: 258.8929x; 258.8929x over previous
"""PhaseSimilarity Trainium2 kernel.

out[b,h,t,a] = where(valid, |sum_n exp(i*(q[b,t,n] - k[b,h,t,a,n]))| / N, -inf)

Strategy (8 NeuronCores, shard (b, t-quarter)):
  per core: 8 heads x 512 t x 64 a x 64 n = 16.7M k-elements (64 MiB fp32).
  1. gpsimd cast-DMA k fp32(HBM) -> fp16(SBUF), natural (t, (a n)) layout.
  2. xbar DMA-transpose -> (p=(a%2,n), J=a//2, t): contraction dim n on
     partitions.
  3. fp16 turns-domain range reduction on DVE (Sin on ScalarE needs
     [-pi,pi]): r' = k/(2pi) - q/(2pi); frac via magic-number rounding;
     separate quarter-shifted rounding for the cos path. Signs are free
     (real/imag get squared).
  4. ScalarE Sin -> cd, sd (fp16)  [the compute bottleneck: 2 passes].
  5. TensorE reduces over n: lhsT = cd-tile (128, t), rhs = 2-col parity
     selector -> PSUM (t, a) in natural output layout, accumulated across
     J with has_written semantics.
  6. Epilogue (batched at the end, 2 ACT table switches total):
     mag = Sqrt((r^2+i^2)/N^2), out = select(valid, mag, -inf).
"""
import numpy as np

B, H, T, A, N = 2, 8, 2048, 64, 64
NCORES = 8
TQS = 4                # t-quarters
TQ = T // TQS          # 512 t per core
PT = 128               # t-chunk (partition dim)
NCH = TQ // PT         # 4 chunks per core
NJ = A // 2            # 32 a-pairs
FREE = A * N           # 4096

INV2PI = 1.0 / (2 * np.pi)
MAGIC = 1536.0         # 1.5 * 2^10: fp16 round-to-nearest-int magic
TWOPI_S = 6.2831       # slightly < 2pi: |scale*frac| strictly inside pi

_CACHE = {}


def _build(loop_n: int = 1):
    import os
    PROBE = os.environ.get("PROBE", "")
    from contextlib import ExitStack

    import concourse.bacc as bacc
    import concourse.bass as bass
    import concourse.tile as tile
    from concourse import mybir

    f16 = mybir.dt.float16
    f32 = mybir.dt.float32
    u8 = mybir.dt.uint8
    SIN = mybir.ActivationFunctionType.Sin
    SQUARE = mybir.ActivationFunctionType.Square
    SQRT = mybir.ActivationFunctionType.Sqrt
    SUB = mybir.AluOpType.subtract
    MULT = mybir.AluOpType.mult
    ISGE = mybir.AluOpType.is_ge

    nc = bacc.Bacc(
        "TRN2", target_bir_lowering=False, debug=False,
        enable_asserts=False, num_devices=NCORES,
    )
    # k arrives pre-cast (fp16) and pre-transposed on host:
    # [ch, h, p=(a%2)*64+n, J=a//2, t]
    k_d = nc.dram_tensor("k", [NCH, H, 128, NJ, PT], f16,
                         kind="ExternalInput").ap()
    q_d = nc.dram_tensor("q", [TQ, N], f32, kind="ExternalInput").ap()
    v_d = nc.dram_tensor("valid", [H, TQ, A], u8, kind="ExternalInput").ap()
    o_d = nc.dram_tensor("out", [H, TQ, A], f32, kind="ExternalOutput").ap()

    with tile.TileContext(nc) as tc, ExitStack() as ctx:
        # load/trig tiles double-buffered for cross-stage overlap; the
        # DVE-internal chain temps (rp, r, rc) are single-buffered (DVE is
        # serial with itself anyway).
        work = ctx.enter_context(tc.tile_pool(name="work", bufs=2))
        loadp = ctx.enter_context(tc.tile_pool(name="loadp", bufs=3))
        chain = ctx.enter_context(tc.tile_pool(name="chain", bufs=1))
        qpool = ctx.enter_context(tc.tile_pool(name="qp", bufs=2))
        acc = ctx.enter_context(tc.tile_pool(name="acc", bufs=1))
        psum = ctx.enter_context(tc.tile_pool(name="psum", bufs=4, space="PSUM"))

        # cos-path bias: Sin(-S*t + S/4)
        cbias = acc.tile([128, 1], f32)
        nc.vector.memset(cbias, TWOPI_S / 4)

        # parity selector: rows 0-63 -> col 0, rows 64-127 -> col 1
        sel = acc.tile([128, 2], f16)
        nc.vector.memset(sel, 0.0)
        nc.vector.memset(sel[0:64, 0:1], 1.0)
        nc.vector.memset(sel[64:128, 1:2], 1.0)

        # per-core accumulation buffers (128, NCH, H, A)
        rbuf = acc.tile([PT, NCH, H, A], f32)
        ibuf = acc.tile([PT, NCH, H, A], f32)
        mbuf = acc.tile([PT, NCH, H, A], u8)

        loop_cm = tc.For_i(0, loop_n, 1) if loop_n > 1 else None
        if loop_cm is not None:
            ctx.enter_context(loop_cm)

        for ch in range(NCH):
            t0 = ch * PT
            # q/(2pi) - MAGICless prep: load fp32, scale, duplicate, transpose
            qf32 = qpool.tile([PT, N], f32, tag="qf32")
            nc.sync.dma_start(out=qf32, in_=q_d[t0:t0 + PT, :])
            qf = qpool.tile([PT, 2, N], f16, tag="qf")
            nc.vector.tensor_scalar_mul(qf[:, 0, :], qf32, INV2PI)
            nc.vector.tensor_scalar_mul(qf[:, 1, :], qf32, INV2PI)
            qT = qpool.tile([128, PT], f16, tag="qT")
            nc.sync.dma_start_transpose(qT, qf.rearrange("t j n -> t (j n)"))
            # materialize q/(2pi) replicated over J (dense step-1 operand so
            # the stt ops hit the fp16 2x perf mode); reused across all 8 h.
            qrep = qpool.tile([128, NJ, PT], f16, tag="qrep")
            nc.vector.tensor_copy(qrep[:, 0, :], qT)
            ncopied = 1
            while ncopied < NJ:
                step = min(ncopied, NJ - ncopied)
                nc.vector.tensor_copy(
                    qrep[:, ncopied:ncopied + step, :], qrep[:, 0:step, :])
                ncopied += step

            for h in range(H):
                kT = loadp.tile([128, NJ, PT], f16, tag="kT")
                nc.sync.dma_start(out=kT, in_=k_d[ch, h])

                # range reduction (turns domain, fp16)
                rp = chain.tile([128, NJ, PT], f16, tag="rp")
                nc.vector.scalar_tensor_tensor(rp, kT, INV2PI, qrep,
                                               op0=MULT, op1=SUB)
                r = chain.tile([128, NJ, PT], f16, tag="r")
                nc.vector.tensor_scalar_add(r, rp, MAGIC)
                mfrac = work.tile([128, NJ, PT], f16, tag="mfrac")
                nc.vector.scalar_tensor_tensor(mfrac, r, MAGIC, rp,
                                               op0=SUB, op1=SUB)
                # cos arg: t = (mfrac >= 1/4) - mfrac; cos(d) = Sin(-S*t + S/4)
                fc = work.tile([128, NJ, PT], f16, tag="fc")
                nc.vector.scalar_tensor_tensor(fc, mfrac, 0.25, mfrac,
                                               op0=ISGE, op1=SUB)

                # trig (ScalarE): sd = -sin(d), cd = -cos(d); signs cancel
                sd = work.tile([128, NJ, PT], f16, tag="sd")
                cd = work.tile([128, NJ, PT], f16, tag="cd")
                if PROBE == "noact":
                    sd, cd = mfrac, fc
                else:
                    nc.scalar.activation(sd, mfrac, SIN, scale=TWOPI_S)
                    nc.scalar.activation(cd, fc, SIN, scale=-TWOPI_S, bias=cbias)

                # reduce over n: PSUM (t, a)
                ps_r = psum.tile([PT, A], f32, tag="ps_r")
                ps_i = psum.tile([PT, A], f32, tag="ps_i")
                NJ_MM = 1 if PROBE == "fewmm" else NJ
                for J in range(NJ_MM):
                    nc.tensor.matmul(ps_r[:, 2 * J:2 * J + 2], cd[:, J, :],
                                     sel, start=(J == 0), stop=(J == NJ_MM - 1))
                for J in range(NJ_MM):
                    nc.tensor.matmul(ps_i[:, 2 * J:2 * J + 2], sd[:, J, :],
                                     sel, start=(J == 0), stop=(J == NJ_MM - 1))

                nc.scalar.copy(rbuf[:, ch, h, :], ps_r)
                nc.scalar.copy(ibuf[:, ch, h, :], ps_i)
                nc.sync.dma_start(out=mbuf[:, ch, h, :],
                                  in_=v_d[h, t0:t0 + PT, :])

        # epilogue: mag = sqrt((r^2 + i^2)/N^2); out = valid ? mag : -inf
        # squares in-place (elementwise stream, writes trail reads)
        nc.scalar.activation(rbuf, rbuf, SQUARE)
        nc.scalar.activation(ibuf, ibuf, SQUARE)
        ss = acc.tile([PT, NCH, H, A], f32)
        nc.vector.tensor_add(ss, rbuf, ibuf)
        nc.scalar.activation(ss, ss, SQRT, scale=1.0 / (N * N))
        res = acc.tile([PT, NCH, H, A], f32)
        nc.vector.memset(res, float("-inf"))
        nc.vector.copy_predicated(res, mbuf, ss)
        for ch in range(NCH):
            for h in range(H):
                nc.sync.dma_start(out=o_d[h, ch * PT:(ch + 1) * PT, :],
                                  in_=res[:, ch, h, :])
    nc.compile()
    return nc


def make_in_maps(q_angles, k_angles, valid):
    q_angles = np.asarray(q_angles, dtype=np.float32)
    k16 = np.asarray(k_angles).astype(np.float16)
    valid_u8 = np.asarray(valid).astype(np.uint8)

    in_maps = []
    for c in range(NCORES):
        b, tq = divmod(c, TQS)
        sl = slice(tq * TQ, (tq + 1) * TQ)
        # [h, t, a, n] -> [ch, h, (a%2)*64+n, a//2, t]
        ks = k16[b, :, sl].reshape(H, NCH, PT, NJ, 2, N)
        kt = np.ascontiguousarray(ks.transpose(1, 0, 4, 5, 3, 2))
        in_maps.append({
            "k": kt.reshape(NCH, H, 128, NJ, PT),
            "q": np.ascontiguousarray(q_angles[b, sl]),
            "valid": np.ascontiguousarray(valid_u8[b, :, sl]),
        })
    return in_maps


def kernel(q_angles, k_angles, valid, batch, time):
    from concourse.bass_utils import run_bass_kernel_spmd

    if "nc" not in _CACHE:
        _CACHE["nc"] = _build()
    nc = _CACHE["nc"]

    in_maps = make_in_maps(q_angles, k_angles, valid)

    res = run_bass_kernel_spmd(nc, in_maps, core_ids=list(range(NCORES)))
    out = np.empty((B, H, T, A), dtype=np.float32)
    for c in range(NCORES):
        b, tq = divmod(c, TQS)
        out[b, :, tq * TQ:(tq + 1) * TQ] = res.results[c]["out"]
    return out
